# revision 25
# baseline (speedup 1.0000x reference)
"""Masked-attention kernel for 8 TRN2 NeuronCores (batch-parallel sharding).

Per-core shard: 2 batches of [S=2048, D=128] Q/K/V + [S, S] bool mask.

Design (v2 — split-fp8 QK with fused mask bias):
  - Scores per k-tile are computed transposed (sc[k, q]) by TWO fp8
    DoubleRow matmuls at 0.5 cyc/col each (1.0 cyc/col total, the same
    PE cost as the old fp16 QK, but the mask now rides along free):
      DR1: lhsT=[Kh|Kl],    rhs=[Qh|Qh] (stride-0 plane broadcast)
      DR2: lhsT=[-224*I|Kh], rhs=[m|Ql]  (m in {0,4} -> bias -896)
    where Qh=fp8(Q), Ql=fp8(Q-Qh), same for K.  The only dropped term
    is Kl*Ql (~0.1% of a score sigma); measured score rel-err 7e-5.
  - The -896 bias replaces all per-tile mask work: no PE mask matmuls,
    no DVE not-mask multiplies, no NMT stream.  Act exp underflows to
    +0; the DVE fast-exp's fp32->int16 conversion saturates at -32768,
    which bitcasts to fp16 -0.0 (verified on HW), so masked lanes
    contribute exactly nothing to PV and the denominator.
  - The per-k-tile [Ql|m] planes come from one packed SBUF tile
    [128, 17, 1024] (slots 0-15 = transposed masks, slot 16 = Ql);
    a step-slice t[:, kt:17:16-kt, n:n+512] selects planes {kt, 16}.
  - exp split for engine balance: 13 tiles on Act (~1.04us each),
    3 fast-exp tiles on DVE (bits = sc*A + B, int16, bitcast fp16).
  - Softmax denominator: 12 tiles chain-added on DVE, 4 tiles
    side-summed on the otherwise-idle Pool engine; 16 one-column PE
    matmuls vs a ones vector (8 on each partial sum) give per-q
    denominators; DVE reciprocal applied after the epilogue transpose.
  - PV: V tile stationary fp16, exp output moving, o^T accumulated in
    PSUM across k-tiles with a 3-tile lag.
  - Chunk epilogues (denominator, PSUM drain, transposes, scales,
    store) are deferred into the next chunk's first k-iterations, and
    the PV tail slides after the next chunk's first QKs, so neither PE
    nor Act drains at a chunk boundary.
  - The loop (timing) build software-pipelines the iteration seam:
    chunk-0 startup tiles (packed mask+Ql, Qh) are persistent, primed
    pre-loop and refilled during the last chunk's prefetch slots.
  - Output is written fp16 in a packed [qc, p, t, d] layout; the host
    unpacks to [S, D] and casts to fp32.
"""

import numpy as np
import ml_dtypes

B, S, D = 16, 2048, 128
NCORES = 8
BP = B // NCORES  # batches per core
P = 128
QC = 1024  # q-chunk (columns of the transposed score tile)
NQC = S // QC
NKT = S // P  # k tiles
NQS = QC // P  # q subtiles per chunk
HKT = NKT // 2  # k tiles per half-load
SCALE = 1.0 / float(np.sqrt(128.0))
MASKW = -224.0  # mask-plane weight; mask values are 4.0 -> bias -896
                # (both within the fp8 e4m3 max-normal of 240)
PVLAG = 3
# exp split: DVE fast-exp tiles (rest go to Act)
FE_TILES = frozenset([3, 7, 11, 15])
# denominator side-sum tiles handled by the Pool engine
POOL_TILES = (0, 2, 4, 6, 8)
# fp16 fast-exp: bits(exp(x)) ~= x*2^10/ln2 + (15*2^10 - 45.5); the
# mult folds in the 1/sqrt(dk) score scale
FEXP_A = 1477.3197 * SCALE
FEXP_B = 15314.5
# last-chunk tile order: fast-exp (DVE) tiles first, Act tiles last,
# so the post-loop flush is never gated on the serial DVE chain
ORDER_LAST = list(range(NKT))

_CACHE = {}


def build_nc(loop=True, nbody=1):
    import concourse.mybir as mybir
    import concourse.tile as tile
    from concourse import bacc

    fp16 = mybir.dt.float16
    fp8 = mybir.dt.float8e4

    nc = bacc.Bacc("TRN2", target_bir_lowering=False, debug=False,
                   num_devices=NCORES)

    QHd = nc.dram_tensor("QH", [BP, D, S], fp8, kind="ExternalInput")
    QLd = nc.dram_tensor("QL", [BP, D, S], fp8, kind="ExternalInput")
    MTd = nc.dram_tensor("MT", [BP, NKT * P, S], fp8, kind="ExternalInput")
    KW1d = nc.dram_tensor("KW1", [BP, D, NKT * 2 * P], fp8,
                          kind="ExternalInput")
    KW2d = nc.dram_tensor("KW2", [BP, D, NKT * 2 * P], fp8,
                          kind="ExternalInput")
    Vd = nc.dram_tensor("V", [BP, P, NKT, D], fp16, kind="ExternalInput")
    if loop:
        Id = nc.dram_tensor("iters", [1, 1], mybir.dt.int32,
                            kind="ExternalInput")
    Od = nc.dram_tensor("out", [BP, NQC, P, NQS, D], fp16,
                        kind="ExternalOutput")

    ident_dram = nc.inline_tensor(np.eye(P, dtype=np.float16),
                                  name="ident_const")

    with tile.TileContext(nc) as tc:
        with tc.tile_pool(name="consts", bufs=1) as consts, \
             tc.tile_pool(name="kwp", bufs=1) as kwp, \
             tc.tile_pool(name="qhp", bufs=2) as qhp, \
             tc.tile_pool(name="mqlp", bufs=2) as mqlp, \
             tc.tile_pool(name="pp", bufs=8) as pp, \
             tc.tile_pool(name="accp", bufs=2) as accp, \
             tc.tile_pool(name="paccp", bufs=2) as paccp, \
             tc.tile_pool(name="outp", bufs=2) as outp, \
             tc.tile_pool(name="spsum", bufs=4, space="PSUM") as spsum, \
             tc.tile_pool(name="opsum", bufs=1, space="PSUM") as opsum, \
             tc.tile_pool(name="tpsum", bufs=1, space="PSUM") as tpsum:

            ident = consts.tile([P, P], fp16)
            nc.gpsimd.dma_start(out=ident[:, :], in_=ident_dram.ap())
            ones_col = consts.tile([P, 1], fp16)
            nc.vector.memset(ones_col, 1.0)

            # persistent chunk-0 startup tiles: primed here, refilled
            # during the last chunk so the next loop iteration's first
            # matmul is gated only by compute
            pmql = consts.tile([P, NKT + 1, QC], fp8)
            pqh = consts.tile([P, QC], fp8)

            def load_prime_mql():
                # Ql + first two mask slots first so the cold-start QK
                # isn't gated on the full 2MB mask stream
                nc.sync.dma_start(out=pmql[:, NKT, :],
                                  in_=QLd.ap()[0, :, 0:QC])
                nc.sync.dma_start(
                    out=pmql[:, 0:2, :],
                    in_=MTd.ap()[0, 0:2 * P, 0:QC]
                        .rearrange("(t p) q -> p t q", t=2))
                nc.sync.dma_start(
                    out=pmql[:, 2:NKT, :],
                    in_=MTd.ap()[0, 2 * P:, 0:QC]
                        .rearrange("(t p) q -> p t q", t=NKT - 2))

            def load_prime_qh():
                nc.scalar.dma_start(out=pqh[:, :], in_=QHd.ap()[0, :, 0:QC])

            load_prime_mql()
            load_prime_qh()

            prime = (pmql, pqh)
            pools = (kwp, qhp, mqlp, pp, accp, paccp, outp,
                     spsum, opsum, tpsum)
            if loop:
                it_sb = consts.tile([1, 1], mybir.dt.int32)
                nc.sync.dma_start(out=it_sb[:, :], in_=Id.ap())
                n_iters = nc.values_load(it_sb[:, :],
                                         skip_runtime_bounds_check=True)
                with tc.For_i(0, n_iters, 1,
                              hint_engines=(mybir.EngineType.PE,
                                            mybir.EngineType.Activation,
                                            mybir.EngineType.DVE,
                                            mybir.EngineType.SP,
                                            mybir.EngineType.Pool)):
                    _kernel_body(nc, mybir, QHd, QLd, MTd, KW1d, KW2d,
                                 Vd, Od, ident, ones_col, prime,
                                 (load_prime_mql, load_prime_qh), *pools)
            else:
                for nb_i in range(nbody):
                    # pass the prime reloaders on all but the last body
                    # so multi-body builds model the loop seam
                    lp = ((load_prime_mql, load_prime_qh)
                          if nb_i + 1 < nbody else None)
                    _kernel_body(nc, mybir, QHd, QLd, MTd, KW1d, KW2d,
                                 Vd, Od, ident, ones_col, prime,
                                 lp, *pools)
    nc.compile()
    return nc


def _kernel_body(nc, mybir, QHd, QLd, MTd, KW1d, KW2d, Vd, Od,
                 ident, ones_col, prime, load_prime,
                 kwp, qhp, mqlp, pp, accp, paccp, outp,
                 spsum, opsum, tpsum):
    fp16 = mybir.dt.float16
    fp32 = mybir.dt.float32
    fp8 = mybir.dt.float8e4
    i16 = mybir.dt.int16
    Exp = mybir.ActivationFunctionType.Exp
    DR = mybir.MatmulPerfMode.DoubleRow

    def load_kw(b):
        t1 = kwp.tile([P, NKT, 2, P], fp8, name=f"kw1_{b}")
        nc.scalar.dma_start(
            out=t1[:, :, :, :],
            in_=KW1d.ap()[b, :, :].rearrange(
                "d (t two k) -> d t two k", t=NKT, two=2))
        t2 = kwp.tile([P, NKT, 2, P], fp8, name=f"kw2_{b}")
        nc.scalar.dma_start(
            out=t2[:, :, :, :],
            in_=KW2d.ap()[b, :, :].rearrange(
                "d (t two k) -> d t two k", t=NKT, two=2))
        return t1, t2

    def load_v_half(b, h):
        t = kwp.tile([P, HKT, D], fp16, name=f"v{b}{h}")
        nc.scalar.dma_start(
            out=t[:, :, :], in_=Vd.ap()[b, :, h * HKT:(h + 1) * HKT, :])
        return t

    def load_mql(b, qc):
        t = mqlp.tile([P, NKT + 1, QC], fp8, name="mql")
        nc.sync.dma_start(
            out=t[:, 0:NKT, :],
            in_=MTd.ap()[b, :, qc * QC:(qc + 1) * QC]
                .rearrange("(t p) q -> p t q", t=NKT))
        nc.sync.dma_start(out=t[:, NKT, :],
                          in_=QLd.ap()[b, :, qc * QC:(qc + 1) * QC])
        return t

    def load_qh(b, qc):
        t = qhp.tile([P, QC], fp8, name="qh")
        nc.scalar.dma_start(out=t[:, :],
                            in_=QHd.ap()[b, :, qc * QC:(qc + 1) * QC])
        return t

    pmql, pqh = prime
    mql_next = {(0, 0): pmql}
    qh_next = {(0, 0): pqh}
    kw = {0: load_kw(0)}
    vv = {0: [load_v_half(0, 0), load_v_half(0, 1)]}

    pend = {}
    pend_pv = []

    def prhs(t, sl):
        ap = t[:, sl]
        return ap.bitcast(fp16) if t.dtype == i16 else ap

    def emit_pv(ops, pts, jj, vh0, vh1, first, final):
        vsel = vh0 if jj < HKT else vh1
        for n in range(0, QC, 512):
            nc.tensor.matmul(
                ops[:, n:n + 512],
                lhsT=vsel[:, jj % HKT, :],
                rhs=prhs(pts[jj], slice(n, n + 512)),
                start=first, stop=final,
                skip_group_check=True)
        del pts[jj]

    def epi_den(c):
        acc, pacc, _, b, qc = pend[c]
        # fold the Pool partial sum in first; single-shot den matmuls
        # (interleaved start/stop pairs on tiny PSUM regions do not
        # accumulate reliably)
        nc.vector.tensor_add(out=acc[:, :], in0=acc[:, :],
                             in1=pacc[:, :])
        den = tpsum.tile([P, NQS], fp32, name="den")
        for sq in range(NQS):
            nc.tensor.matmul(den[:, sq:sq + 1],
                             lhsT=acc[:, sq * P:(sq + 1) * P],
                             rhs=ones_col[:, :],
                             start=True, stop=True,
                             skip_group_check=True)
        rcol = outp.tile([P, NQS], fp32, name="rcol")
        nc.vector.reciprocal(out=rcol[:, :], in_=den[:, :])
        pend[c] += (rcol,)

    def epi_copy(c, last=False):
        _, _, ops, b, qc, _ = pend[c]
        # PSUM drain (GPSIMD cannot access PSUM on real HW); in the
        # final flush Act is already idle, so it takes half
        ot = outp.tile([P, QC], fp16, name="ot")
        if last:
            H = QC // 2
            nc.vector.tensor_copy(out=ot[:, :H], in_=ops[:, :H])
            nc.scalar.copy(out=ot[:, H:], in_=ops[:, H:])
        else:
            nc.vector.tensor_copy(out=ot[:, :], in_=ops[:, :])
        pend[c] += (ot,)

    def epi_out(c, last=False):
        _, _, _, b, qc, rcol, ot = pend.pop(c)
        osb = tpsum.tile([P, QC], fp16, name="osb")
        osf = outp.tile([P, NQS, D], fp16, name="osf")
        HQ = NQS // 2
        for hh in range(2):
            for t in range(hh * HQ, (hh + 1) * HQ):
                nc.tensor.transpose(osb[:, t * P:(t + 1) * P],
                                    ot[:, t * P:(t + 1) * P],
                                    ident[:, :])
            for t in range(hh * HQ, (hh + 1) * HQ):
                if last and t % 2 == 1:
                    nc.scalar.activation(
                        out=osf[:, t, :],
                        in_=osb[:, t * P:(t + 1) * P],
                        func=mybir.ActivationFunctionType.Copy,
                        scale=rcol[:, t:t + 1])
                else:
                    nc.vector.tensor_scalar_mul(
                        out=osf[:, t, :],
                        in0=osb[:, t * P:(t + 1) * P],
                        scalar1=rcol[:, t:t + 1])
            ring = (nc.sync.dma_start if (hh == 0 or last)
                    else nc.gpsimd.dma_start)
            ring(out=Od.ap()[b, qc, :, hh * HQ:(hh + 1) * HQ, :],
                 in_=osf[:, hh * HQ:(hh + 1) * HQ, :])

    for b in range(BP):
        for qc in range(NQC):
            c = b * NQC + qc
            kw1t, kw2t = kw[b]
            vh0, vh1 = vv[b]
            qh = qh_next.pop((b, qc))
            mql = mql_next.pop((b, qc))
            if qc + 1 < NQC:
                nb, nqc = b, qc + 1
            elif b + 1 < BP:
                nb, nqc = b + 1, 0
            else:
                nb = None
            acc = accp.tile([P, QC], fp16, name="acc")
            pacc = paccp.tile([P, QC], fp16, name="pacc")
            ops = opsum.tile([P, QC], fp32, name="opsum")
            pts = {}
            ndve = 0
            npool = 0
            dve_first = None
            pool_first = None
            # last chunk: fast-exp tiles first so the final serial
            # flush (PV tail, den, drain) never waits on the DVE chain
            order = (ORDER_LAST if nb is None else list(range(NKT)))
            for i in range(NKT):
                kt = order[i]
                # scores in two 512-col half tiles (1 PSUM bank each, 4
                # rotating buffers) so the PE can run two full k-tiles
                # ahead of the exp consumers
                sch = []
                for n in range(0, QC, 512):
                    sc = spsum.tile([P, 512], fp32, name="sch")
                    sch.append(sc)
                    # QK DR1: [Kh|Kl] x [Qh|Qh]
                    nc.tensor.matmul(
                        sc[:, :],
                        lhsT=kw1t[:, kt, :, :],
                        rhs=qh[:, n:n + 512].unsqueeze(1)
                            .broadcast_to([P, 2, 512]),
                        start=True, stop=False,
                        perf_mode=DR, skip_group_check=True)
                    # QK DR2: [Kh|-448I] x [Ql|m] -> + Kh*Ql + mask
                    nc.tensor.matmul(
                        sc[:, :],
                        lhsT=kw2t[:, kt, :, :],
                        rhs=mql[:, kt:NKT + 1:NKT - kt, n:n + 512],
                        start=False, stop=True,
                        perf_mode=DR, skip_group_check=True)

                # previous chunk's PV tail + deferred epilogue, placed
                # AFTER this kt's QK so the exp pipeline never bubbles
                if pend_pv:
                    if i == 0:
                        emit_pv(*pend_pv.pop(0))
                        emit_pv(*pend_pv.pop(0))
                    elif i == 1:
                        emit_pv(*pend_pv.pop(0))
                if c - 1 in pend:
                    if i == 1:
                        epi_den(c - 1)
                    elif i == 2:
                        epi_copy(c - 1)
                    elif i == 4:
                        epi_out(c - 1)

                if kt in FE_TILES:
                    # DVE fast-exp: int16(x*A + B) bitcast to fp16;
                    # masked lanes saturate to -32768 == fp16 -0.0
                    fe = pp.tile([P, QC], i16, name="fe")
                    for hn, n in enumerate(range(0, QC, 512)):
                        nc.vector.tensor_scalar(
                            out=fe[:, n:n + 512], in0=sch[hn][:, :],
                            scalar1=FEXP_A, scalar2=FEXP_B,
                            op0=mybir.AluOpType.mult,
                            op1=mybir.AluOpType.add)
                    pts[kt] = fe
                else:
                    # one full-width Act exp (amortizes the PSUM access
                    # latency); reads both half tiles via their fixed
                    # rotation adjacency is not guaranteed, so emit two
                    # halves only when the pool split them apart
                    pt = pp.tile([P, QC], fp16, name="pt")
                    for hn, n in enumerate(range(0, QC, 512)):
                        nc.scalar.activation(out=pt[:, n:n + 512],
                                             in_=sch[hn][:, :],
                                             func=Exp, scale=SCALE)
                    pts[kt] = pt

                # denominator accumulation: Pool side-chain for
                # POOL_TILES, DVE chain for the rest
                if kt in POOL_TILES:
                    npool += 1
                    if npool == 1:
                        pool_first = kt
                    elif npool == 2:
                        nc.gpsimd.tensor_add(
                            out=pacc[:, :],
                            in0=prhs(pts[pool_first], slice(None)),
                            in1=prhs(pts[kt], slice(None)))
                    else:
                        nc.gpsimd.tensor_add(
                            out=pacc[:, :], in0=pacc[:, :],
                            in1=prhs(pts[kt], slice(None)))
                else:
                    ndve += 1
                    if ndve == 1:
                        dve_first = kt
                    elif ndve == 2:
                        nc.vector.tensor_add(
                            out=acc[:, :],
                            in0=prhs(pts[dve_first], slice(None)),
                            in1=prhs(pts[kt], slice(None)))
                    else:
                        nc.vector.tensor_add(
                            out=acc[:, :], in0=acc[:, :],
                            in1=prhs(pts[kt], slice(None)))

                # prefetches (after compute emission so they never gate
                # it).  The loop-seam prime refills are issued one chunk
                # BEFORE the last chunk so the ~6us mask stream lands
                # before the next iteration's first QK needs it.
                if i == 3:
                    if nb is not None:
                        mql_next[(nb, nqc)] = load_mql(nb, nqc)
                    elif load_prime is not None:
                        load_prime[0]()
                if i == 6:
                    if nb is not None:
                        qh_next[(nb, nqc)] = load_qh(nb, nqc)
                    elif load_prime is not None:
                        load_prime[1]()
                if nb is not None and nqc == 0:
                    if i == 8:
                        kw[nb] = load_kw(nb)
                    elif i == 10:
                        vv[nb] = [load_v_half(nb, 0)]
                    elif i == 12:
                        vv[nb].append(load_v_half(nb, 1))

                # PV lags PVLAG k-tiles so the PE never waits on exp
                if i >= PVLAG:
                    emit_pv(ops, pts, order[i - PVLAG], vh0, vh1,
                            first=(i == PVLAG), final=False)
            tail = list(order[NKT - PVLAG:])
            for x, jj in enumerate(tail):
                pend_pv.append((ops, pts, jj, vh0, vh1, False,
                                x == len(tail) - 1))
            pend[c] = (acc, pacc, ops, b, qc)

    # final flush (no next chunk to hide it in); all epilogue compute
    # stays OFF the Act engine so its in-order queue flows straight
    # into the next loop iteration's first exps
    while pend_pv:
        emit_pv(*pend_pv.pop(0))
    c = BP * NQC - 1
    epi_den(c)
    epi_copy(c, last=True)
    epi_out(c, last=True)


def _get_nc(loop=False):
    key = f"nc_loop{loop}"
    if key not in _CACHE:
        _CACHE[key] = build_nc(loop=loop)
    return _CACHE[key]


def make_in_maps(Q, K, V, mask):
    """Host-side shard + layout prep: per-core input dicts."""
    fp8 = ml_dtypes.float8_e4m3
    Q = np.asarray(Q, dtype=np.float32)
    K = np.asarray(K, dtype=np.float32)
    V = np.asarray(V, dtype=np.float32)
    mask_b = np.asarray(mask).astype(bool)
    eye = np.eye(P, dtype=np.float32)
    in_maps = []
    for c in range(NCORES):
        sl = slice(c * BP, (c + 1) * BP)
        qt = np.ascontiguousarray(Q[sl].transpose(0, 2, 1))  # [BP, D, S]
        kt = np.ascontiguousarray(K[sl].transpose(0, 2, 1))  # [BP, D, S]
        qh = qt.astype(fp8)
        ql = (qt - qh.astype(np.float32)).astype(fp8)
        kh = kt.astype(fp8)
        kl = (kt - kh.astype(np.float32)).astype(fp8)
        # per-k-tile stationary pairs [BP, D, NKT, 2, P]
        kw1 = np.empty((BP, D, NKT, 2, P), fp8)
        kw2 = np.empty((BP, D, NKT, 2, P), fp8)
        kh4 = kh.reshape(BP, D, NKT, P)
        kl4 = kl.reshape(BP, D, NKT, P)
        kw1[:, :, :, 0, :] = kh4
        kw1[:, :, :, 1, :] = kl4
        # DR2 rhs planes are (mask slot kt, Ql slot 16) in that order,
        # so plane 0 of the stationary is the -448*I mask weight and
        # plane 1 is Kh
        kw2[:, :, :, 0, :] = (MASKW * eye)[None, :, None, :].astype(fp8)
        kw2[:, :, :, 1, :] = kh4
        # V packed partition-major: [BP, P, NKT, D]
        v16 = np.ascontiguousarray(
            V[sl].reshape(BP, NKT, P, D).transpose(0, 2, 1, 3)
        ).astype(np.float16)
        # transposed mask, masked entries = 2.0 (bias = 2 * -448)
        mT = np.ascontiguousarray(mask_b[sl].transpose(0, 2, 1))
        mt8 = (4.0 * mT.reshape(BP, NKT * P, S)).astype(fp8)
        in_maps.append({
            "QH": qh, "QL": ql,
            "KW1": np.ascontiguousarray(kw1.reshape(BP, D, NKT * 2 * P)),
            "KW2": np.ascontiguousarray(kw2.reshape(BP, D, NKT * 2 * P)),
            "MT": mt8, "V": v16,
        })
    return in_maps


def unpack_out(raw):
    """[BP, NQC, P, NQS, D] fp16 -> [BP, S, D] fp32."""
    return np.ascontiguousarray(
        raw.transpose(0, 1, 3, 2, 4)).reshape(BP, S, D).astype(np.float32)


def kernel(Q, K, V, mask, dk=128):
    from concourse.bass_utils import run_bass_kernel_spmd

    assert int(dk) == 128
    nc = _get_nc(loop=False)
    in_maps = make_in_maps(Q, K, V, mask)
    res = run_bass_kernel_spmd(nc, in_maps, core_ids=list(range(NCORES)))
    return np.concatenate([unpack_out(r["out"]) for r in res.results],
                          axis=0)
